# revision 23
# baseline (speedup 1.0000x reference)
"""Causal multi-head attention layer for Trainium2 (Bass/Tile), 8 NeuronCores.

Problem: x[B=2,S=2048,D=1024], H=16 heads, Dh=64.
Sharding: data-parallel over batch (2) x tensor-parallel over head groups (4):
each of the 8 cores handles one batch element and 4 heads, producing a partial
output [S, D]; the host sums the 4 head-group partials per batch (the
"all-reduce after the W_O contraction" done host-side since we return full
output anyway) and adds biases that commute out (b_O and sum_h b_V[h] @ W_O[h],
exact because softmax rows sum to 1).

Device kernel (per core), all operands resident in SBUF:
  - x^T is fed pre-transposed from host: [128, KT=8, S] (D on partitions).
  - Q^T, K^T computed head-PAIR-packed: [128, NPAIR, S] (partitions 0:64 =
    head 2*pr dims, 64:128 = head 2*pr+1). W as stationary [128,128], x^T
    moving N=512.
  - V computed in [k, e] layout (x^T stationary, W_V moving N=256, all 4
    heads at once) and stored with an appended ones-column: V' = [V | 1].
  - Scores computed TRANSPOSED: S^T[k, q] = (K^T tile).T @ Q^T chunk, so
    softmax's sum lands on the matmul contraction instead of needing row
    reductions: Z'[e|1, q] = V'.T @ exp(S^T) accumulated over k-tiles gives
    both the unnormalized attention output (rows 0:64) and the softmax
    denominator l (row 64) in one accumulation. No max-subtraction is needed:
    scores are O(1) here, exp is safe in fp32.
  - Causal mask applied multiplicatively on exp(S^T) for diagonal chunks only.
  - 1/l broadcast across partitions via a K=2 matmul outer product, then
    Z^T normalized on DVE; output projection accumulates head pairs in PSUM.
The head-pair packing puts the two heads' K=64 contractions on disjoint PE
row groups (base partition 0 / 64), so their matmuls run concurrently.
"""

import os
import numpy as np

# 'fp32r' = fp32 bits, single-pass reduced-precision PE mode (full speed)
# 'bf16'  = bf16 storage/matmuls (full speed, lower precision)
# 'fp32'  = exact fp32 matmuls (4x slower on PE)
MM_MODE = os.environ.get("ATTN_MM_MODE", "fp32r")

P = 128
SC = 512  # q-chunk width (one PSUM bank of fp32)

_BUILD_CACHE = {}


def _np_sb(mm_mode):
    if mm_mode == "bf16":
        import ml_dtypes

        return np.dtype(ml_dtypes.bfloat16)
    return np.dtype(np.float32)


def build_nc(S, Dm, NH, Dh, mm_mode, stage=99):
    """Build (and cache) the per-core Bass module. NH = heads per core.

    stage: debug truncation — 1=projections only, 2=+scores/exp, 3=+PV,
    4=+normalize, 99=full kernel.
    """
    key = (S, Dm, NH, Dh, mm_mode, stage)
    if key in _BUILD_CACHE:
        return _BUILD_CACHE[key]

    import concourse.bacc as bacc
    import concourse.mybir as mybir
    import concourse.tile as tile

    f32 = mybir.dt.float32
    # dt_w: dtype of every matmul operand. float32r data is fp32 bits that the
    # PE consumes in a single-pass reduced-precision mode; the BIR verifier
    # requires every fp32r matmul operand to be *produced* with float32r dtype
    # (DMA pass-through from a float32r DRAM tensor, or a compute-engine write).
    dt_w = {
        "bf16": mybir.dt.bfloat16,
        "fp32": mybir.dt.float32,
        "fp32r": mybir.dt.float32r,
    }[mm_mode]
    # dtype for the tiny 1/l broadcast matmul operands (always reduced-precision
    # single-pass unless running the exact-fp32 fallback)
    dt_r = mybir.dt.float32 if mm_mode == "fp32" else mybir.dt.float32r
    # dtype for non-matmul elementwise tiles (masks)
    dt_m = mybir.dt.bfloat16 if mm_mode == "bf16" else mybir.dt.float32

    KT = Dm // P       # k-tiles over the model dim (contraction of projections)
    NPAIR = NH // 2    # head pairs
    QC = S // SC       # q chunks
    NKT = S // P       # k-position tiles
    DH2 = Dm // SC     # output free-dim halves
    assert Dh == 64 and NH % 2 == 0 and S % SC == 0 and Dm % SC == 0

    nc = bacc.Bacc(
        "TRN2",
        debug=False,
        enable_asserts=False,
        target_bir_lowering=False,
        num_devices=1,
    )

    xT_d = nc.dram_tensor("xT", [P, KT, S], dt_w, kind="ExternalInput")
    wqk_d = nc.dram_tensor("wqk", [P, KT, 2, NPAIR, P], dt_w, kind="ExternalInput")
    wv_d = nc.dram_tensor("wv", [P, KT, NH * Dh], dt_w, kind="ExternalInput")
    wo_d = nc.dram_tensor("wo", [P, NPAIR, Dm], dt_w, kind="ExternalInput")
    bqk_d = nc.dram_tensor("bqk", [P, 2, NPAIR], f32, kind="ExternalInput")
    out_d = nc.dram_tensor("out", [S, Dm], f32, kind="ExternalOutput")

    def mm(ap):
        return ap

    mm32 = mm

    Exp = mybir.ActivationFunctionType.Exp
    inv_sqrt_dh = 1.0 / float(np.sqrt(Dh))

    with tile.TileContext(nc) as tc:
        with (
            tc.tile_pool(name="const", bufs=1) as cpool,
            tc.tile_pool(name="psmm", bufs=5, space="PSUM") as psmm,
            tc.tile_pool(name="psz", bufs=2, space="PSUM") as psz,
        ):
            # ---------- constants ----------
            wv = cpool.tile([P, KT, NH * Dh], dt_w)
            nc.sync.dma_start(wv[:], wv_d[:])
            wo = cpool.tile([P, NPAIR, Dm], dt_w)
            nc.sync.dma_start(wo[:], wo_d[:])
            bqk = cpool.tile([P, 2, NPAIR], f32)
            nc.sync.dma_start(bqk[:], bqk_d[:])

            # causal masks for the 4 diagonal-chunk variants: keep (1.0) where
            # q >= k + v*128, else 0.0  (S^T layout: partition=k, free=q)
            masks = cpool.tile([P, SC // P, SC], dt_m)
            nc.gpsimd.memset(masks[:], 1.0)
            for v in range(SC // P):
                nc.gpsimd.affine_select(
                    out=masks[:, v, :],
                    in_=masks[:, v, :],
                    compare_op=mybir.AluOpType.is_ge,
                    fill=0.0,
                    base=-(v * P),
                    pattern=[[1, SC]],
                    channel_multiplier=-1,
                )

            # ones row for the 1/l partition-broadcast outer product (K=1 matmul)
            ones64 = cpool.tile([1, 64], dt_r)

            QTt = cpool.tile([P, NPAIR, S], dt_w)
            KTt = cpool.tile([P, NPAIR, S], dt_w)
            # V' = [V | 1 | 0]: ones column feeds the softmax denominator into
            # the PV accumulation; zero pad keeps the stationary free dim even
            # (fp32r matmul restriction).
            Vt = cpool.tile([P, NKT, NH, Dh + 2], dt_w)

            # ---------- phase 1: projections ----------
            with tc.tile_pool(name="p1", bufs=1) as p1pool:
                wqk = p1pool.tile([P, KT, 2, NPAIR, P], dt_w)
                nc.sync.dma_start(wqk[:], wqk_d[:])
                xT = p1pool.tile([P, KT, S], dt_w)
                for kt in range(KT):
                    nc.sync.dma_start(xT[:, kt, :], xT_d[:, kt, :])

                # memset can't write float32r: stage constants in f32, copy over
                cstage = p1pool.tile([P, NKT, NH, 2], f32)
                nc.vector.memset(cstage[:, :, :, 0:1], 1.0)
                nc.vector.memset(cstage[:, :, :, 1:2], 0.0)
                nc.vector.tensor_copy(Vt[:, :, :, Dh : Dh + 2], cstage[:])
                ostage = p1pool.tile([1, 64], f32)
                nc.vector.memset(ostage[:], 1.0)
                nc.vector.tensor_copy(ones64[:], ostage[:])

                for pr in range(NPAIR):
                    for qc in range(QC):
                        psQ = psmm.tile([P, SC], f32, tag="mm")
                        psK = psmm.tile([P, SC], f32, tag="mm")
                        for kt in range(KT):
                            st, sp = kt == 0, kt == KT - 1
                            xs = xT[:, kt, qc * SC : (qc + 1) * SC]
                            nc.tensor.matmul(
                                psQ[:], mm(wqk[:, kt, 0, pr, :]), mm(xs),
                                start=st, stop=sp,
                            )
                            nc.tensor.matmul(
                                psK[:], mm(wqk[:, kt, 1, pr, :]), mm(xs),
                                start=st, stop=sp,
                            )
                        qsl = QTt[:, pr, qc * SC : (qc + 1) * SC]
                        ksl = KTt[:, pr, qc * SC : (qc + 1) * SC]
                        nc.vector.tensor_scalar_add(qsl, psQ[:], bqk[:, 0, pr : pr + 1])
                        nc.vector.tensor_scalar_add(ksl, psK[:], bqk[:, 1, pr : pr + 1])

                for qt in range(NKT):
                    psV = psmm.tile([P, NH * Dh], f32, tag="mm")
                    for kt in range(KT):
                        nc.tensor.matmul(
                            psV[:],
                            mm(xT[:, kt, qt * P : (qt + 1) * P]),
                            mm(wv[:, kt, :]),
                            start=(kt == 0), stop=(kt == KT - 1),
                        )
                    nc.vector.tensor_copy(
                        Vt[:, qt, :, 0:Dh],
                        psV[:].rearrange("p (h e) -> p h e", e=Dh),
                    )

            # ---------- phases 2+3 ----------
            with (
                tc.tile_pool(name="zt", bufs=1) as ztpool,
                tc.tile_pool(name="e", bufs=6) as epool,
                tc.tile_pool(name="r", bufs=2) as rpool,
                tc.tile_pool(name="o", bufs=3) as opool,
            ):
                ZTt = ztpool.tile([P, NPAIR, S], dt_w)

                if stage <= 1:
                    nc.sync.dma_start(out_d[0:P, :], QTt[:, 0, 0:Dm])

                # flash attention, scores transposed
                for pr in range(NPAIR if stage >= 2 else 0):
                    hA, hB = 2 * pr, 2 * pr + 1
                    for qc in range(QC):
                        zA = psz.tile([Dh + 2, SC], f32, tag="z")
                        zB = psz.tile([Dh + 2, SC], f32, tag="z")
                        jmax = (qc + 1) * (SC // P)
                        qs = slice(qc * SC, (qc + 1) * SC)
                        pend = None

                        def emit_pv(j, eA, eB, jmax=jmax, zA=zA, zB=zB):
                            st, sp = j == 0, j == jmax - 1
                            nc.tensor.matmul(
                                zA[:], mm(Vt[:, j, hA, :]), mm(eA[:]),
                                start=st, stop=sp,
                            )
                            nc.tensor.matmul(
                                zB[:], mm(Vt[:, j, hB, :]), mm(eB[:]),
                                start=st, stop=sp,
                            )

                        for j in range(jmax):
                            sA = psmm.tile([P, SC], f32, tag="mm")
                            sB = psmm.tile([P, SC], f32, tag="mm")
                            ks = slice(j * P, (j + 1) * P)
                            nc.tensor.matmul(
                                sA[:], mm(KTt[0:64, pr, ks]), mm(QTt[0:64, pr, qs]),
                                start=True, stop=True,
                            )
                            nc.tensor.matmul(
                                sB[:], mm(KTt[64:128, pr, ks]), mm(QTt[64:128, pr, qs]),
                                start=True, stop=True,
                            )
                            eA = epool.tile([P, SC], dt_w, tag="e")
                            eB = epool.tile([P, SC], dt_w, tag="e")
                            nc.scalar.activation(eA[:], sA[:], Exp, scale=inv_sqrt_dh)
                            nc.scalar.activation(eB[:], sB[:], Exp, scale=inv_sqrt_dh)
                            v = j - (jmax - SC // P)
                            if v >= 0:  # chunk contains the causal diagonal
                                nc.vector.tensor_mul(eA[:], eA[:], masks[:, v, :])
                                nc.vector.tensor_mul(eB[:], eB[:], masks[:, v, :])
                            if stage >= 3:
                                if pend is not None:
                                    emit_pv(*pend)
                                pend = (j, eA, eB)
                            else:
                                last_e = eA
                        if stage < 3:
                            if pr == 0 and qc == 0:
                                nc.sync.dma_start(out_d[0:P, 0:SC], last_e[:])
                            continue
                        emit_pv(*pend)
                        if stage < 4:
                            if pr == 0 and qc == 0:
                                nc.sync.dma_start(out_d[0 : Dh + 2, 0:SC], zA[:])
                            continue

                        # normalize: ZT[:, q] = Z'[0:64, q] / Z'[64, q]
                        rA = rpool.tile([1, SC], dt_r, tag="r")
                        rB = rpool.tile([1, SC], dt_r, tag="r")
                        with nc.allow_low_precision("fp32r reciprocal for PE broadcast"):
                            nc.vector.reciprocal(rA[:], zA[Dh : Dh + 1, :])
                            nc.vector.reciprocal(rB[:], zB[Dh : Dh + 1, :])
                        bcA = psmm.tile([64, SC], f32, tag="mm")
                        bcB = psmm.tile([64, SC], f32, tag="mm")
                        nc.tensor.matmul(
                            bcA[:], mm32(ones64[:]), mm32(rA[:]), start=True, stop=True
                        )
                        nc.tensor.matmul(
                            bcB[:], mm32(ones64[:]), mm32(rB[:]), start=True, stop=True
                        )
                        # DVE reads at most one PSUM operand: stage z in SBUF
                        tzA = rpool.tile([Dh, SC], f32, tag="tz")
                        tzB = rpool.tile([Dh, SC], f32, tag="tz")
                        nc.vector.tensor_copy(tzA[:], zA[0:Dh, :])
                        nc.vector.tensor_copy(tzB[:], zB[0:Dh, :])
                        nc.vector.tensor_mul(ZTt[0:64, pr, qs], tzA[:], bcA[:])
                        nc.vector.tensor_mul(ZTt[64:128, pr, qs], tzB[:], bcB[:])

                if stage == 4:
                    nc.sync.dma_start(out_d[0:P, :], ZTt[:, 0, 0:Dm])

                # output projection: out[q, d] = sum_h Z_h[q, :] @ W_O[h]
                for t in range(S // P if stage >= 5 else 0):
                    for dh2 in range(DH2):
                        po = psmm.tile([P, SC], f32, tag="mm")
                        ds = slice(dh2 * SC, (dh2 + 1) * SC)
                        zs = slice(t * P, (t + 1) * P)
                        # K=128 contraction sums the head pair in one matmul
                        for pr in range(NPAIR):
                            nc.tensor.matmul(
                                po[:], mm(ZTt[:, pr, zs]), mm(wo[:, pr, ds]),
                                start=(pr == 0), stop=(pr == NPAIR - 1),
                            )
                        ot = opool.tile([P, SC], f32, tag="o")
                        nc.vector.tensor_copy(ot[:], po[:])
                        nc.sync.dma_start(out_d[t * P : (t + 1) * P, ds], ot[:])

    nc.compile()
    _BUILD_CACHE[key] = nc
    return nc


def pack_inputs(x_b, W_Q, W_K, W_V, W_O, b_Q, b_K, hds, mm_mode):
    """Host-side packing of one core's shard into the kernel's layouts."""
    npdt = _np_sb(mm_mode)
    Dm, Dh = W_Q.shape[1], W_Q.shape[2]
    S = x_b.shape[0]
    NH = len(hds)
    NPAIR = NH // 2
    KT = Dm // P

    xT = np.ascontiguousarray(
        x_b.T.reshape(KT, P, S).transpose(1, 0, 2)
    ).astype(npdt)

    def pack_w_in(W):  # [H, Dm, Dh] -> [P, KT, NPAIR, 2*Dh]
        W4 = np.asarray(W)[hds]  # [NH, Dm, Dh]
        t = W4.reshape(NPAIR, 2, KT, P, Dh).transpose(3, 2, 0, 1, 4)
        return t.reshape(P, KT, NPAIR, 2 * Dh)

    wqk = np.ascontiguousarray(
        np.stack([pack_w_in(W_Q), pack_w_in(W_K)], axis=2)  # [P, KT, 2, NPAIR, 128]
    ).astype(npdt)

    WV4 = np.asarray(W_V)[hds]  # [NH, Dm, Dh]
    wv = np.ascontiguousarray(
        WV4.reshape(NH, KT, P, Dh).transpose(2, 1, 0, 3).reshape(P, KT, NH * Dh)
    ).astype(npdt)

    WO4 = np.asarray(W_O)[hds]  # [NH, Dh, Dm]
    wo = np.ascontiguousarray(
        WO4.reshape(NPAIR, 2, Dh, Dm).transpose(1, 2, 0, 3).reshape(P, NPAIR, Dm)
    ).astype(npdt)

    def pack_b(b):  # [H, Dh] -> [P, NPAIR]
        b4 = np.asarray(b)[hds]
        return b4.reshape(NPAIR, 2, Dh).transpose(1, 2, 0).reshape(P, NPAIR)

    bqk = np.ascontiguousarray(
        np.stack([pack_b(b_Q), pack_b(b_K)], axis=1)  # [P, 2, NPAIR]
    ).astype(np.float32)

    return {"xT": xT, "wqk": wqk, "wv": wv, "wo": wo, "bqk": bqk}


def kernel(x, W_Q, W_K, W_V, W_O, b_Q, b_K, b_V, b_O, _trace=False):
    from concourse.bass_utils import run_bass_kernel_spmd

    x = np.asarray(x, np.float32)
    B, S, Dm = x.shape
    H, _, Dh = W_Q.shape
    NCORES = 8
    GB = NCORES // B        # head groups per batch element
    NH = H // GB            # heads per core

    nc = build_nc(S, Dm, NH, Dh, MM_MODE)

    in_maps = []
    for c in range(NCORES):
        b, g = c // GB, c % GB
        hds = list(range(g * NH, (g + 1) * NH))
        in_maps.append(
            pack_inputs(x[b], W_Q, W_K, W_V, W_O, b_Q, b_K, hds, MM_MODE)
        )

    res = run_bass_kernel_spmd(
        nc, in_maps, core_ids=list(range(NCORES)), trace=_trace
    )

    out = np.zeros((B, S, Dm), np.float32)
    for c in range(NCORES):
        out[c // GB] += res.results[c]["out"]

    # biases that commute out of the device kernel (softmax rows sum to 1)
    corr = np.asarray(b_O, np.float32) + np.einsum(
        "he,hed->d",
        np.asarray(b_V, np.float32),
        np.asarray(W_O, np.float32),
    )
    out += corr[None, None, :]

    if _trace:
        kernel.last_results = res
    return out
